# revision 2
# baseline (speedup 1.0000x reference)
"""ConvQRNN Trainium2 kernel.

Strategy (8 NeuronCores, spatial H-sharding, 8 rows/core):
  - Conv3d(k=(2,3,3), CIN=3 -> 256) lowered to matmul: host builds a fp16
    im2col with K=56 rows (54 taps + ones row carrying the conv bias + one
    zero pad row).  Per timestep each gate's [64ch x 2048pix] pre-activation
    is computed as two col-tiled matmuls (duplicated weight halves) so the
    PSUM layout directly matches the scan layout.
  - Scan layout "B": every scan tensor is [128, 1024] fp16 with
    partition = (b//2)*64 + ch, free = (b%2)*512 + h*64 + w.
  - QRNN cell runs fp16 on DVE/ACT; o-gate/sigmoid/tanh tail is batched over
    8 timesteps; H is written to DRAM as fp16 and upcast on the host.
"""

import os

import numpy as np

B, CIN, T, H, W = 4, 3, 32, 64, 64
COUT = 64
NC = 8
HS = H // NC
K = 56  # 54 conv taps + ones(bias) row + zero pad row
PIX = B * HS * W          # 2048 pixels per core per timestep
F = PIX // 2              # 1024 free elements per tile
KAPPA = 8                 # o-gate tail batch (timesteps)

f16 = np.float16

_CACHE = {}

LAST_RESULTS = {}


def _host_prep(X, Wconv, bconv, W_ci, W_cf, W_co):
    X = np.ascontiguousarray(np.asarray(X, np.float32))
    Wconv = np.asarray(Wconv, np.float32)
    bconv = np.asarray(bconv, np.float32)
    Xp = np.pad(X, ((0, 0), (0, 0), (1, 0), (1, 1), (1, 1)))  # (B,CIN,T+1,H+2,W+2)

    im2col = np.zeros((NC, K, T, PIX), f16)
    for c in range(NC):
        for cin in range(CIN):
            for dt in range(2):
                for dh in range(3):
                    for dw in range(3):
                        k = ((cin * 2 + dt) * 3 + dh) * 3 + dw
                        blk = Xp[:, cin, dt:dt + T,
                                 8 * c + dh:8 * c + dh + HS, dw:dw + W]
                        blk = blk.reshape(2, 2, T, HS, W).transpose(2, 0, 1, 3, 4)
                        im2col[c, k] = blk.reshape(T, PIX).astype(f16)
        im2col[c, 54] = 1.0

    lhsT = np.zeros((4, K, 128), f16)
    Wr = Wconv.reshape(4, COUT, CIN, 2, 3, 3)
    for g in range(4):
        wk = Wr[g].transpose(1, 2, 3, 4, 0).reshape(54, COUT).astype(f16)
        lhsT[g, :54, :64] = wk
        lhsT[g, :54, 64:] = wk
        lhsT[g, 54, :64] = bconv[g * 64:(g + 1) * 64].astype(f16)
        lhsT[g, 54, 64:] = bconv[g * 64:(g + 1) * 64].astype(f16)

    peep = np.zeros((NC, 3, 128, F), f16)
    for i, Wc in enumerate((W_ci, W_cf, W_co)):
        Wc = np.asarray(Wc, np.float32)
        for c in range(NC):
            sl = Wc[:, 8 * c:8 * c + HS, :].reshape(64, HS * W).astype(f16)
            tile = np.empty((128, F), f16)
            for half in range(2):
                for b1 in range(2):
                    tile[64 * half:64 * half + 64, 512 * b1:512 * b1 + 512] = sl
            peep[c, i] = tile
    return im2col, lhsT, peep


def _build_nc(loop_reps=1):
    import concourse.bacc as bacc
    import concourse.mybir as mybir
    from contextlib import nullcontext
    from concourse.tile import TileContext

    fp16 = mybir.dt.float16
    fp32 = mybir.dt.float32
    AF = mybir.ActivationFunctionType

    nc = bacc.Bacc(None, target_bir_lowering=False)

    im2col_d = nc.dram_tensor("im2col", [K, T, PIX], fp16, kind="ExternalInput")
    lhsT_d = nc.dram_tensor("lhsT", [4, K, 128], fp16, kind="ExternalInput")
    peep_d = nc.dram_tensor("peep", [3, 128, F], fp16, kind="ExternalInput")
    out_d = nc.dram_tensor("out", [T, 128, F], fp16, kind="ExternalOutput")

    with TileContext(nc) as tc:
        with (
            tc.tile_pool(name="const", bufs=1) as constp,
            tc.tile_pool(name="state", bufs=1) as statep,
            tc.tile_pool(name="rhs", bufs=3) as rhsp,
            tc.tile_pool(name="work", bufs=2) as workp,
            tc.tile_pool(name="tail", bufs=1) as tailp,
            tc.tile_pool(name="psum", bufs=1, space="PSUM") as psump,
        ):
            wci = constp.tile([128, F], fp16)
            wcf = constp.tile([128, F], fp16)
            wco = constp.tile([128, F], fp16)
            nc.sync.dma_start(out=wci[:], in_=peep_d[0])
            nc.sync.dma_start(out=wcf[:], in_=peep_d[1])
            nc.sync.dma_start(out=wco[:], in_=peep_d[2])
            lhsT_sb = constp.tile([K, 4 * 128], fp16)
            nc.sync.dma_start(
                out=lhsT_sb[:].rearrange("k (g m) -> k g m", g=4),
                in_=lhsT_d[:].rearrange("g k m -> k g m"),
            )

            # C ring: slot s holds C_{8k+s-1}; slot 0 seeded with zeros /
            # previous window's last state.
            c_hist = statep.tile([128, (KAPPA + 1) * F], fp16)
            a_o_hist = statep.tile([128, KAPPA * F], fp16)
            nc.vector.memset(c_hist[:, 0:F], 0.0)

            e_if = psump.tile([128, 2 * F], fp32)
            e_g = psump.tile([128, F], fp32)
            e_o = psump.tile([128, F], fp32)

            loop_cm = tc.For_i(0, loop_reps) if loop_reps > 1 else nullcontext()
            with loop_cm:
                for t in range(T):
                        j = t % KAPPA
                        c_prev = c_hist[:, j * F:(j + 1) * F]
                        c_next = c_hist[:, (j + 1) * F:(j + 2) * F]

                        rhs = rhsp.tile([K, PIX], fp16)
                        nc.sync.dma_start(out=rhs[:], in_=im2col_d[:, t, :])

                        # gates: i -> e_if[:, 0:F], f -> e_if[:, F:2F], g, o
                        for g, (ptile, foff) in enumerate(
                            ((e_if, 0), (e_if, F), (e_g, 0), (e_o, 0))
                        ):
                            for hf in range(2):
                                lw = lhsT_sb[:, g * 128 + 64 * hf:g * 128 + 64 * hf + 64]
                                for q in range(2):
                                    nc.tensor.matmul(
                                        ptile[64 * hf:64 * hf + 64,
                                              foff + 512 * q:foff + 512 * q + 512],
                                        lw,
                                        rhs[:, 1024 * hf + 512 * q:1024 * hf + 512 * q + 512],
                                        start=True,
                                        stop=True,
                                        tile_position=(0, 64 * hf),
                                    )

                        v_if = workp.tile([128, 2 * F], fp16)
                        nc.vector.tensor_mul(out=v_if[:, 0:F], in0=wci[:], in1=c_prev)
                        nc.vector.tensor_mul(out=v_if[:, F:2 * F], in0=wcf[:], in1=c_prev)
                        a_if = workp.tile([128, 2 * F], fp16)
                        nc.vector.tensor_add(out=a_if[:], in0=e_if[:], in1=v_if[:])
                        s_if = workp.tile([128, 2 * F], fp16)
                        nc.scalar.activation(s_if[:], a_if[:], AF.Sigmoid)
                        t_g = workp.tile([128, F], fp16)
                        nc.scalar.activation(t_g[:], e_g[:], AF.Tanh)
                        p1 = workp.tile([128, F], fp16)
                        nc.vector.tensor_mul(out=p1[:], in0=s_if[:, 0:F], in1=t_g[:])
                        p2 = workp.tile([128, F], fp16)
                        nc.vector.tensor_mul(out=p2[:], in0=s_if[:, F:2 * F], in1=c_prev)
                        nc.vector.tensor_add(out=c_next, in0=p1[:], in1=p2[:])

                        v_o = workp.tile([128, F], fp16)
                        nc.vector.tensor_mul(out=v_o[:], in0=wco[:], in1=c_next)
                        nc.vector.tensor_add(
                            out=a_o_hist[:, j * F:(j + 1) * F], in0=e_o[:], in1=v_o[:]
                        )

                        if j == KAPPA - 1:
                            s_o = tailp.tile([128, KAPPA * F], fp16)
                            nc.scalar.activation(s_o[:], a_o_hist[:], AF.Sigmoid)
                            t_c = tailp.tile([128, KAPPA * F], fp16)
                            nc.scalar.activation(t_c[:], c_hist[:, F:(KAPPA + 1) * F], AF.Tanh)
                            h8 = tailp.tile([128, KAPPA * F], fp16)
                            nc.vector.tensor_mul(out=h8[:], in0=s_o[:], in1=t_c[:])
                            k0 = t - KAPPA + 1
                            nc.sync.dma_start(
                                out=out_d[k0:k0 + KAPPA].rearrange("t p f -> p t f"),
                                in_=h8[:].rearrange("p (t f) -> p t f", t=KAPPA),
                            )
                            if t != T - 1:
                                # carry last state of the window into ring slot 0
                                nc.vector.tensor_copy(
                                    out=c_hist[:, 0:F],
                                    in_=c_hist[:, KAPPA * F:(KAPPA + 1) * F],
                                )

    nc.compile()
    return nc


def _get_nc():
    if "nc" not in _CACHE:
        _CACHE["nc"] = _build_nc()
    return _CACHE["nc"]


def make_in_maps(X, Wconv, bconv, W_ci, W_cf, W_co):
    im2col, lhsT, peep = _host_prep(X, Wconv, bconv, W_ci, W_cf, W_co)
    return [
        {"im2col": im2col[c], "lhsT": lhsT, "peep": peep[c]} for c in range(NC)
    ]


def kernel(X, Wconv, bconv, W_ci, W_cf, W_co):
    from concourse.bass_utils import run_bass_kernel_spmd

    nc = _get_nc()
    in_maps = make_in_maps(X, Wconv, bconv, W_ci, W_cf, W_co)
    trace = bool(os.environ.get("QRNN_TRACE"))
    res = run_bass_kernel_spmd(
        nc, in_maps, core_ids=list(range(NC)), trace=trace
    )
    LAST_RESULTS["exec_time_ns"] = getattr(res, "exec_time_ns", None)

    O = np.empty((B, COUT, T, H, W), np.float32)
    for c in range(NC):
        o = np.asarray(res.results[c]["out"], f16).astype(np.float32)
        o = o.reshape(T, 2, 64, 2, HS, W).transpose(1, 3, 2, 0, 4, 5)
        O[:, :, :, 8 * c:8 * c + HS, :] = o.reshape(B, COUT, T, HS, W)
    return O

